# revision 1
# baseline (speedup 1.0000x reference)
"""Depthwise 3x3 conv on 8 trn2 NeuronCores — v4 (channel-major).

Host transposes x to [b, c, h*w] fp16. On device, partitions = channels.
For each tap (dh, dw), PE multiplies by diag(W[tap, c]) and accumulates in
PSUM: the per-channel weighting AND the 9-tap sum both happen on the PE.
(h, w) shifts are free-dim AP offsets into a zero-padded 114x114 pixel grid
(padding written by a strided DMA into a pre-zeroed tile).

c=192 = 128 + 64: per image pair (A, B), three 128-partition tiles:
  g=0: A channels 0..127, g=1: B channels 0..127,
  g=2: A channels 128..191 | B channels 128..191 (packed 64+64).

ScalarE evacuates PSUM -> fp16; stores are c-major; host transposes back.
"""
import dataclasses

import numpy as np

import concourse.bacc as bacc
import concourse.mybir as mybir
from concourse.bass_utils import run_bass_kernel_spmd
from concourse.tile import TileContext

F32 = mybir.dt.float32
F16 = mybir.dt.float16
NPF16 = np.float16

B, H, W, C = 32, 112, 112, 192
N_CORES = 8
B_SH = B // N_CORES
NPIX = H * W                 # 12544
PW = W + 2                   # padded row length 114
NPAD = (H + 2) * PW          # 12996
RCH = 4                      # output rows per PSUM chunk
PCH = RCH * W                # 448 fp32 per chunk
NCH = H // RCH               # 28 chunks
QCH = 7                      # chunks per store tile
QFREE = QCH * PCH            # 3136


def _ap3(t, offset, s0, n0, s1, n1):
    """3D free AP [[s0,n0],[s1,n1]] at free offset within tile t."""
    sl = t[:, offset:offset + 1]
    return dataclasses.replace(sl, ap=[sl.ap[0], [s0, n0], [s1, n1]])


def _load_spread(nc, g, x_a, x_b, spool, xps):
    """Contiguous DMA into a staging tile, then DVE-spread into padded grid.

    The strided (224B-descriptor) direct load is slow; a contiguous load
    plus a 2x-mode DVE copy with a row-gap output AP is much faster, and
    the DVE is otherwise idle in this kernel.
    """
    srcs = {0: [(x_a, 0, 128, 0)],
            1: [(x_b, 0, 128, 0)],
            2: [(x_a, 128, 64, 0), (x_b, 128, 64, 64)]}[g]
    xt = spool.tile([128, NPIX], F16, tag="xt", name="xt")
    for xs, c0, ncp, p0 in srcs:
        nc.sync.dma_start(out=xt[p0:p0 + ncp, :], in_=xs[c0:c0 + ncp, :])
    nc.vector.tensor_scalar_add(
        _ap3(xps[g], PW + 1, PW, H, 1, W),
        xt[:, :].rearrange("c (h w) -> c h w", w=W), 0.0)


def _emit_conv_g(nc, wd, xps, opool, pspool, g, y_a, y_b):
    """Conv + evac + store for one 128-partition channel tile g."""
    gsel = 1 if g == 2 else 0
    for q in range(NCH // QCH):
        outq = opool.tile([128, QFREE], F16, tag="outq", name="outq")
        for cc in range(QCH):
            ch = q * QCH + cc
            r0 = ch * RCH
            ps = pspool.tile([128, PCH], F32, tag="ps", name="ps")
            for t in range(9):
                dh, dw = divmod(t, 3)
                off = (r0 + dh) * PW + dw
                nc.tensor.matmul(
                    ps[:, :], wd[:, (t * 2 + gsel) * 128:
                                 (t * 2 + gsel + 1) * 128],
                    _ap3(xps[g], off, PW, RCH, 1, W),
                    start=(t == 0), stop=(t == 8))
            nc.scalar.activation(outq[:, cc * PCH:(cc + 1) * PCH],
                                 ps[:, :],
                                 mybir.ActivationFunctionType.Copy)
        q0 = q * QFREE
        if g < 2:
            y = y_a if g == 0 else y_b
            nc.scalar.dma_start(out=y[0:128, q0:q0 + QFREE],
                                in_=outq[:, :])
        else:
            nc.scalar.dma_start(out=y_a[128:192, q0:q0 + QFREE],
                                in_=outq[0:64, :])
            nc.scalar.dma_start(out=y_b[128:192, q0:q0 + QFREE],
                                in_=outq[64:128, :])


def _alloc_xpads(nc, xpool):
    xps = [xpool.tile([128, NPAD], F16, tag=f"xp_{g}", name=f"xp_{g}")
           for g in range(3)]
    for xp in xps:
        # only the guard cells need zeroing; the interior is DVE-written
        nc.vector.memset(xp[:, 0:PW], 0.0)                  # top row
        nc.vector.memset(xp[:, (H + 1) * PW:NPAD], 0.0)    # bottom row
        nc.vector.memset(_ap3(xp, PW, PW, H, 1, 1), 0.0)   # left col
        nc.vector.memset(_ap3(xp, 2 * PW - 1, PW, H, 1, 1), 0.0)  # right
    return xps


def _build_module(b_sh=B_SH, opts=None):
    opts = dict(_DEFAULT_OPTS, **(opts or {}))
    nc = bacc.Bacc("TRN2")
    x = nc.dram_tensor("x", [b_sh, C, NPIX], F16, kind="ExternalInput")
    wdiag = nc.dram_tensor("wdiag", [128, 20 * 128], F16,
                           kind="ExternalInput")
    y = nc.dram_tensor("y", [b_sh, C, NPIX], F16, kind="ExternalOutput")

    with TileContext(nc) as tc:
        with (
            tc.tile_pool(name="const", bufs=1) as cpool,
            tc.tile_pool(name="xp", bufs=1) as xpool,
            tc.tile_pool(name="stg", bufs=2) as spool,
            tc.tile_pool(name="outp", bufs=3) as opool,
            tc.tile_pool(name="psum", bufs=4, space="PSUM") as pspool,
        ):
            wd = cpool.tile([128, 20 * 128], F16, tag="wd", name="wd")
            nc.sync.dma_start(out=wd[:, :], in_=wdiag[:, :])
            xps = _alloc_xpads(nc, xpool)
            npair = b_sh // 2
            for g in range(3):
                _load_spread(nc, g, x[0], x[1], spool, xps)
            for pair in range(npair):
                a, b = 2 * pair, 2 * pair + 1
                for g in range(3):
                    _emit_conv_g(nc, wd, xps, opool, pspool, g, y[a], y[b])
                    if pair + 1 < npair:
                        # JIT refill of tile g for the next pair: the DVE
                        # spread waits on PE's last read of xps[g], and runs
                        # under PE's conv of tiles g+1/g+2.
                        _load_spread(nc, g, x[a + 2], x[b + 2], spool, xps)
    nc.compile()
    return nc


def _build_timing_module(iters=8, skip=(), opts=None):
    opts = dict(_DEFAULT_OPTS, **(opts or {}))
    nc = bacc.Bacc("TRN2")
    x = nc.dram_tensor("xg", [2, C, NPIX], F16)
    y = nc.dram_tensor("yg", [2, C, NPIX], F16)
    yo = nc.dram_tensor("yo", [1, 8], F32, kind="ExternalOutput")

    with TileContext(nc) as tc:
        with (
            tc.tile_pool(name="const", bufs=1) as cpool,
            tc.tile_pool(name="xp", bufs=1) as xpool,
            tc.tile_pool(name="stg", bufs=2) as spool,
            tc.tile_pool(name="outp", bufs=3) as opool,
            tc.tile_pool(name="psum", bufs=4, space="PSUM") as pspool,
        ):
            wd = cpool.tile([128, 20 * 128], F16, tag="wd", name="wd")
            nc.vector.memset(wd[:, :], 0.01)
            xps = _alloc_xpads(nc, xpool)
            zt = opool.tile([128, QFREE], F16, tag="outq", name="zt")
            nc.vector.memset(zt[:, :], 0.5)
            for img in range(2):
                for q in range(4):
                    nc.sync.dma_start(
                        out=x[img, 0:128, q * QFREE:(q + 1) * QFREE],
                        in_=zt[:, :])
                    nc.sync.dma_start(
                        out=x[img, 64:192, q * QFREE:(q + 1) * QFREE],
                        in_=zt[:, :])
            for g in range(3):
                _load_spread(nc, g, x[0], x[1], spool, xps)
            with tc.For_i(0, iters) as _:
                # one iter = one image pair (2 images), steady-state JIT refill
                for g in range(3):
                    _emit_conv_g(nc, wd, xps, opool, pspool, g, y[0], y[1])
                    _load_spread(nc, g, x[0], x[1], spool, xps)
            of = opool.tile([1, 8], F32, tag="outq", name="of")
            nc.vector.memset(of[:, :], 0.0)
            nc.sync.dma_start(out=yo[:, :], in_=of[:1, :8])
    nc.compile()
    return nc


def _host_consts(wk, bk):
    """wk [3,3,1,192] -> wdiag [128, 20*128] f16 (18 diag mats + bias)."""
    wd = np.zeros((128, 20 * 128), np.float32)
    for t in range(9):
        dh, dw = divmod(t, 3)
        w_t = wk[dh, dw, 0]  # [192]
        d0 = np.diag(w_t[0:128])
        d1 = np.diag(np.concatenate([w_t[128:192], w_t[128:192]]))
        wd[:, (t * 2) * 128:(t * 2 + 1) * 128] = d0
        wd[:, (t * 2 + 1) * 128:(t * 2 + 2) * 128] = d1
    # bias row selector + bias values (cols 18*128 .. ) — unused when b=0
    return wd.astype(NPF16)


_DEFAULT_OPTS = dict(bias=False)

_NC_CACHE = {}


def kernel(x, w, b):
    x = np.asarray(x, dtype=np.float32)
    wk = np.asarray(w, dtype=np.float32)
    bk = np.asarray(b, dtype=np.float32)
    assert x.shape == (B, H, W, C), x.shape

    if "nc" not in _NC_CACHE:
        _NC_CACHE["nc"] = _build_module()
    nc = _NC_CACHE["nc"]

    xt = np.ascontiguousarray(
        x.astype(NPF16).transpose(0, 3, 1, 2).reshape(B, C, NPIX))
    wdiag = _host_consts(wk, bk)
    in_maps = []
    for core in range(N_CORES):
        sh = xt[core * B_SH:(core + 1) * B_SH]
        in_maps.append({"x": np.ascontiguousarray(sh), "wdiag": wdiag})
    res = run_bass_kernel_spmd(nc, in_maps, core_ids=list(range(N_CORES)))
    out = np.empty((B, C, NPIX), np.float32)
    for core in range(N_CORES):
        out[core * B_SH:(core + 1) * B_SH] = res.results[core]["y"]
    out = np.ascontiguousarray(
        out.reshape(B, C, H, W).transpose(0, 2, 3, 1))
    if np.any(bk != 0.0):
        # conv is linear: bias adds exactly on the host in fp32
        out += bk
    return out

